# revision 1
# baseline (speedup 1.0000x reference)
"""AffineCouplingLayer Trainium2 kernel.

y[:, even] = x[:, even]
y[:, odd]  = x[:, odd] * exp(2*tanh(s)) + t,   [s|t] = MLP(x[:, even])

MLP (compressed-residual): a1 = W0 xp + b0; a2 = W1 elu(a1) + b1;
a3 = W2 elu(a2) + b2; st = W3 elu(a1+a2+a3) + b3.

Strategy: pure data parallelism over rows across 8 NeuronCores. Per core,
process chunks of FD=1024 rows (matmuls split into N=512 halves):
  - DMA x chunk natural layout [128 rows-part, 4*128]
  - PE-transpose 128x128 tiles -> xT [feat, rows] (PSUM), evacuate to SBUF
  - MM1..MM3 with weights stationary, activations transposed [feat, rows]
  - elu(z)+1 = min(exp(z),1) + relu(z)  (exact), z = a_raw + bias:
      exp on ACT (bias via per-partition bias AP),
      relu via tensor_scalar(add bias, max 0),
      join via scalar_tensor_tensor((g min 1) add v)
    The +1 shift is corrected in the next layer's bias (c = b - W.sum(1)).
  - last layer: per-row-tile matmuls with lhsT = e3-tile slices -> ROW-MAJOR
    st tile in PSUM; output bias preloaded via K=1 ones-matmul.
  - tanh/exp(2*.) on s columns; odd columns of the x chunk updated in place;
    store whole chunk.
"""

import os
from contextlib import ExitStack

import numpy as np

import concourse.bass as bass
import concourse.bacc as bacc
import concourse.tile as tile
from concourse import mybir
from concourse.bass_utils import run_bass_kernel_spmd

F32 = mybir.dt.float32
F32R = mybir.dt.float32r
AF = mybir.ActivationFunctionType
OP = mybir.AluOpType

N, D, H = 1048576, 128, 128
P = D // 2
NCORES = 8
FD = 1024         # rows per chunk
NT = FD // 128    # row-tiles per chunk
HF = 512          # matmul half (fp32 moving max)


def build_nc(nr, engine_plan=None, pbufs=(1, 1, 1, 1)):
    """Build the single-core bass program for nr rows."""
    ep = {
        # which engine runs each rebalanceable op: 'act' | 'vec' | 'gp'
        "evac": "split",    # xT PSUM->SBUF copy
        "relu1": "vec",
        "relu2": "vec",
        "relu3": "vec",
        "e1": "vec",
        "e2": "gp",
        "e3": "gp",
        "m1": "gp",
    }
    if engine_plan:
        ep.update(engine_plan)

    nc = bacc.Bacc("TRN2", target_bir_lowering=False, debug=False)

    x = nc.dram_tensor("x", [nr, D], F32, kind="ExternalInput")
    w1t = nc.dram_tensor("w1t", [D, H], F32, kind="ExternalInput")
    w2t = nc.dram_tensor("w2t", [H, H], F32, kind="ExternalInput")
    w3t = nc.dram_tensor("w3t", [H, H], F32, kind="ExternalInput")
    wst = nc.dram_tensor("wst", [H, D], F32, kind="ExternalInput")
    ident = nc.dram_tensor("ident", [128, 128], F32, kind="ExternalInput")
    ocst = nc.dram_tensor("ocst", [1, 128 + FD], F32, kind="ExternalInput")
    bin_c = nc.dram_tensor("bin_c", [H, 1], F32, kind="ExternalInput")
    c2_c = nc.dram_tensor("c2_c", [H, 1], F32, kind="ExternalInput")
    c3res_c = nc.dram_tensor("c3res_c", [H, 1], F32, kind="ExternalInput")
    y = nc.dram_tensor("y", [nr, D], F32, kind="ExternalOutput")

    n_chunks = nr // FD

    with tile.TileContext(nc) as tc, ExitStack() as ctx:
        cpool = ctx.enter_context(tc.tile_pool(name="const", bufs=1))
        W1 = cpool.tile([D, H], F32, tag="W1")
        W2 = cpool.tile([H, H], F32, tag="W2")
        W3 = cpool.tile([H, H], F32, tag="W3")
        WST = cpool.tile([H, D], F32, tag="WST")
        ID = cpool.tile([128, 128], F32, tag="ID")
        OCST = cpool.tile([1, 128 + FD], F32, tag="OCST")
        BIN = cpool.tile([H, 1], F32, tag="BIN")
        C2 = cpool.tile([H, 1], F32, tag="C2")
        CRES = cpool.tile([H, 1], F32, tag="CRES")
        nc.gpsimd.dma_start(W1.bitcast(F32R), w1t[:, :].bitcast(F32R))
        nc.gpsimd.dma_start(W2.bitcast(F32R), w2t[:, :].bitcast(F32R))
        nc.gpsimd.dma_start(W3.bitcast(F32R), w3t[:, :].bitcast(F32R))
        nc.gpsimd.dma_start(WST.bitcast(F32R), wst[:, :].bitcast(F32R))
        nc.gpsimd.dma_start(ID.bitcast(F32R), ident[:, :].bitcast(F32R))
        nc.gpsimd.dma_start(OCST.bitcast(F32R), ocst[:, :].bitcast(F32R))
        nc.gpsimd.dma_start(BIN, bin_c[:, :])
        nc.gpsimd.dma_start(C2, c2_c[:, :])
        nc.gpsimd.dma_start(CRES, c3res_c[:, :])

        xpool = ctx.enter_context(tc.tile_pool(name="xc", bufs=5))
        xtpool = ctx.enter_context(tc.tile_pool(name="xt", bufs=3))
        gpool = ctx.enter_context(tc.tile_pool(name="g", bufs=3))
        vpool = ctx.enter_context(tc.tile_pool(name="v", bufs=3))
        epool = ctx.enter_context(tc.tile_pool(name="e", bufs=3))
        spool = ctx.enter_context(tc.tile_pool(name="s", bufs=3))
        pT = ctx.enter_context(tc.tile_pool(name="pT", bufs=pbufs[0], space="PSUM"))
        pA = ctx.enter_context(tc.tile_pool(name="pA", bufs=pbufs[1], space="PSUM"))
        pB = ctx.enter_context(tc.tile_pool(name="pB", bufs=pbufs[2], space="PSUM"))
        pST = ctx.enter_context(tc.tile_pool(name="pST", bufs=pbufs[3], space="PSUM"))

        # Pre-touch const tiles on PE so steady-state matmuls/transposes never
        # carry const-DMA waits (S3_LW sync slots are scarce: transpose and
        # K=1 matmuls only support a single wait command).
        pre = pT.tile([128, FD], F32, tag="pt")
        nc.tensor.transpose(pre[:, 0:128].bitcast(F32R), ID.bitcast(F32R), ID.bitcast(F32R))
        for Wc in (W1, W2, W3, WST):
            nc.tensor.matmul(
                pre[:, 0:128], Wc.bitcast(F32R), Wc.bitcast(F32R),
                start=True, stop=True, skip_group_check=True,
            )
        scrA = cpool.tile([128, 3], F32, tag="scrA")
        scrD = cpool.tile([128, 3], F32, tag="scrD")
        for i, Bc in enumerate((BIN, C2, CRES)):
            nc.scalar.copy(scrA[:, i : i + 1], Bc)
            nc.vector.tensor_copy(scrD[:, i : i + 1], Bc)

        def relu_biased(engine, out, src, bias_ap):
            # out = max(src + bias, 0)
            eng = {"act": nc.scalar, "vec": nc.vector, "gp": nc.gpsimd}[engine]
            if engine == "act":
                eng.activation(out, src, AF.Relu, bias=bias_ap, scale=1.0)
            else:
                eng.tensor_scalar(out, src, bias_ap, 0.0, OP.add, OP.max)

        def mmr(out, lhsT, rhs, **kw):
            nc.tensor.matmul(out, lhsT.bitcast(F32R), rhs.bitcast(F32R), **kw)

        def elu_join(engine, out, g, v):
            # out = min(g, 1) + v; out tagged f32r for the consuming matmul
            if engine == "gp":
                # Pool codegen lacks scalar_tensor_tensor: clamp in place, add.
                nc.gpsimd.tensor_scalar(g, g, 1.0, None, OP.min)
                nc.gpsimd.tensor_tensor(out.bitcast(F32R), g, v, OP.add)
            else:
                nc.vector.scalar_tensor_tensor(out.bitcast(F32R), g, 1.0, v, OP.min, OP.add)

        for c in range(n_chunks):
            r0 = c * FD
            X = xpool.tile([128, FD], F32, tag="X")
            nc.sync.dma_start(
                X.rearrange("p (t f) -> p t f", t=NT).bitcast(F32R),
                x[r0 : r0 + FD, :].rearrange("(t p) f -> p t f", p=128).bitcast(F32R),
            )

            # --- transpose x chunk: xT[feat, rows]
            pt = pT.tile([128, FD], F32, tag="pt")
            for t in range(NT):
                nc.tensor.transpose(
                    pt[:, 128 * t : 128 * (t + 1)].bitcast(F32R),
                    X[:, 128 * t : 128 * (t + 1)].bitcast(F32R),
                    ID.bitcast(F32R),
                )
            XT = xtpool.tile([128, FD], F32, tag="XT")
            if ep["evac"] == "act":
                nc.scalar.copy(XT.bitcast(F32R), pt)
            elif ep["evac"] == "split":
                nc.scalar.copy(XT[:, : FD // 2].bitcast(F32R), pt[:, : FD // 2])
                nc.vector.tensor_copy(XT[:, FD // 2 :].bitcast(F32R), pt[:, FD // 2 :])
            else:
                nc.vector.tensor_copy(XT.bitcast(F32R), pt)

            # --- layer 1: a1 = W0 @ xpass (transposed, raw)
            a1 = pA.tile([128, FD], F32, tag="a1")
            for h in range(2):
                hs = slice(h * HF, (h + 1) * HF)
                mmr(a1[:, hs], W1, XT[:, hs], start=True, stop=True,
                    skip_group_check=True)
            g1 = gpool.tile([128, FD], F32, tag="g1")
            nc.scalar.activation(g1, a1, AF.Exp, bias=BIN[:, 0:1], scale=1.0)
            v1 = vpool.tile([128, FD], F32, tag="v1")
            relu_biased(ep["relu1"], v1, a1, BIN[:, 0:1])
            e1 = epool.tile([128, FD], F32, tag="e1")
            elu_join(ep["e1"], e1, g1, v1)

            # --- layer 2 (and accumulate a3 into same bank later)
            a2 = pB.tile([128, FD], F32, tag="a2")
            for h in range(2):
                hs = slice(h * HF, (h + 1) * HF)
                mmr(a2[:, hs], W2, e1[:, hs], start=True, stop=True,
                    skip_group_check=True)
            g2 = gpool.tile([128, FD], F32, tag="g2")
            nc.scalar.activation(g2, a2, AF.Exp, bias=C2[:, 0:1], scale=1.0)
            v2 = vpool.tile([128, FD], F32, tag="v2")
            relu_biased(ep["relu2"], v2, a2, C2[:, 0:1])
            e2 = epool.tile([128, FD], F32, tag="e2")
            elu_join(ep["e2"], e2, g2, v2)

            # --- layer 3 + a1 accumulated onto a2 bank: pB = a1+a2+a3 = res
            for h in range(2):
                hs = slice(h * HF, (h + 1) * HF)
                mmr(a2[:, hs], W3, e2[:, hs], start=False, stop=False,
                    skip_group_check=True)
                mmr(a2[:, hs], W1, XT[:, hs], start=False, stop=True,
                    skip_group_check=True)

            g3 = gpool.tile([128, FD], F32, tag="g3")
            nc.scalar.activation(g3, a2, AF.Exp, bias=CRES[:, 0:1], scale=1.0)
            v3 = vpool.tile([128, FD], F32, tag="v3")
            relu_biased(ep["relu3"], v3, a2, CRES[:, 0:1])
            e3 = epool.tile([128, FD], F32, tag="e3")
            elu_join(ep["e3"], e3, g3, v3)

            # --- output layer, ROW-MAJOR per tile; bias preloaded via K=1 mm
            st = pST.tile([128, FD], F32, tag="st")
            for h in range(2):
                mmr(st[:, h * HF : (h + 1) * HF], OCST[:, 0:128],
                    OCST[:, 128 + h * HF : 128 + (h + 1) * HF],
                    start=True, stop=False, skip_group_check=True)
            for t in range(NT):
                mmr(
                    st[:, 128 * t : 128 * (t + 1)],
                    e3[:, 128 * t : 128 * (t + 1)],
                    WST,
                    start=False,
                    stop=(t % 4 == 3),
                    skip_group_check=True,
                )
            st_v = st.rearrange("p (t f) -> p t f", t=NT)

            # --- s chain: e = exp(2*tanh(s))
            th = spool.tile([128, FD // 2], F32, tag="th")
            th_v = th.rearrange("p (t f) -> p t f", t=NT)
            nc.scalar.activation(th_v, st_v[:, :, 0:P], AF.Tanh)
            es = spool.tile([128, FD // 2], F32, tag="es")
            nc.scalar.activation(es, th, AF.Exp, bias=0.0, scale=2.0)
            es_v = es.rearrange("p (t f) -> p t f", t=NT)

            # --- combine into odd columns of X, in place
            X_r = X.rearrange("p (t u two) -> p t u two", t=NT, two=2)
            X_odd = X_r[:, :, :, 1]
            m1 = spool.tile([128, FD // 2], F32, tag="m1")
            m1_v = m1.rearrange("p (t f) -> p t f", t=NT)
            if ep["m1"] == "gp":
                nc.gpsimd.tensor_tensor(m1_v, X_odd, es_v, OP.mult)
            else:
                nc.vector.tensor_tensor(m1_v, X_odd, es_v, OP.mult)
            nc.vector.tensor_tensor(X_odd.bitcast(F32R), m1_v, st_v[:, :, P:D], OP.add)

            nc.sync.dma_start(
                y[r0 : r0 + FD, :].rearrange("(t p) f -> p t f", p=128),
                X.rearrange("p (t f) -> p t f", t=NT),
            )

    nc.compile()
    return nc


def _prep_consts(W_in, b_in, W_b1, b_b1, W_b2, b_b2, W_out, b_out, pass_idx):
    w1t = np.zeros((D, H), dtype=np.float32)
    w1t[np.asarray(pass_idx), :] = W_in.T  # [64,H] scattered into pass rows
    w2t = np.ascontiguousarray(W_b1.T.astype(np.float32))
    w3t = np.ascontiguousarray(W_b2.T.astype(np.float32))
    wst = np.ascontiguousarray(W_out.T.astype(np.float32))
    rs1 = W_b1.sum(axis=1)
    rs2 = W_b2.sum(axis=1)
    rs3 = W_out.sum(axis=1)
    c2 = (b_b1 - rs1).astype(np.float32)
    c3 = (b_b2 - rs2).astype(np.float32)
    cres = (b_in + c2 + c3).astype(np.float32)
    cst = (b_out - rs3).astype(np.float32)
    consts = {
        "w1t": w1t,
        "w2t": w2t,
        "w3t": w3t,
        "wst": wst,
        "ident": np.eye(128, dtype=np.float32),
        "ocst": np.concatenate(
            [np.ones(128, dtype=np.float32), np.tile(cst, FD // D).astype(np.float32)]
        )[None, :],
        "bin_c": b_in.astype(np.float32).reshape(H, 1),
        "c2_c": c2.reshape(H, 1),
        "c3res_c": cres.reshape(H, 1),
    }
    return consts


def kernel(
    x,
    W_in,
    b_in,
    W_b1,
    b_b1,
    W_b2,
    b_b2,
    W_out,
    b_out,
    pass_idx,
    trans_idx,
    _trace=False,
    _trace_kwargs=None,
):
    x = np.ascontiguousarray(np.asarray(x, dtype=np.float32))
    nr = x.shape[0] // NCORES
    consts = _prep_consts(
        np.asarray(W_in, np.float32),
        np.asarray(b_in, np.float32),
        np.asarray(W_b1, np.float32),
        np.asarray(b_b1, np.float32),
        np.asarray(W_b2, np.float32),
        np.asarray(b_b2, np.float32),
        np.asarray(W_out, np.float32),
        np.asarray(b_out, np.float32),
        np.asarray(pass_idx),
    )
    nc = build_nc(nr)
    in_maps = [
        {"x": np.ascontiguousarray(x[i * nr : (i + 1) * nr]), **consts}
        for i in range(NCORES)
    ]
    kw = {}
    if _trace:
        kw = {"trace": True, **(_trace_kwargs or {})}
    res = run_bass_kernel_spmd(nc, in_maps, list(range(NCORES)), **kw)
    out = np.concatenate([res.results[i]["y"] for i in range(NCORES)], axis=0)
    if _trace:
        kernel._last_results = res
    return out

